# revision 26
# baseline (speedup 1.0000x reference)
"""Trainium2 Bass kernel for nn_Att_Bilinear_layer2_keycat_textual_visual.

Math (full shapes B=32,N=64,A=32,O=32,D=512,QD=512):
    v      = einsum('bnao,bod->bnad', att1, obj_reps) + t_rep
    inter  = einsum('bnq,qd->bnd', q[:,:,0,:], W)
    logits = einsum('bnd,bnad->bna', inter, v) + bias
    s      = softmax((logits/t)*m) * m ; att2 = s / (sum_a s + 1e-13)
    out    = einsum('bna,bnao->bno', att2, att1)

Restructured to avoid materializing v (saves ~2/3 of the FLOPs):
    logits[b,n,a] = t_rep[b,n,a,:].inter[b,n,:] + att1[b,n,a,:].s1[b,n,:]
    where s1[b,n,o] = inter[b,n,:].obj_reps[b,o,:]

Sparsity: the masked softmax renormalizes by sum(s), so att2 — and hence
the output — depends ONLY on logits where tags_attention==1 (masked
entries are multiplied by m twice and the softmax denominator cancels in
the renormalization).  tags are ~50% dense, so the host packs, per
32-token group, only the unmasked (n,a) columns of t_rep/att1
consecutively ("CSR-style"), cutting the dominant t_rep HBM stream and
the block-diagonal PE work roughly in half.  Because each token has at
most A=32 unmasked entries, its packed window covers each column residue
mod 32 at most once, so a single strided mod-32 reduce recovers each
token's logits in a per-token ROTATED slot order; the rotation is folded
into the host-built validity mask and the gathered att1 layout, so the
on-device softmax + final einsum are oblivious to it.

Sharding: data-parallel over batch b (4 of 32 per core, 8 cores), W
replicated.  No collectives.  Host-side prep re-lays-out shard bytes
(pack/gather, transposes, fp16 downcast of matmul operands) — all FLOPs
of the reference computation run on-device.

On-device per core (BL=4 batches, TOK=256 tokens, CAP=576 packed cols
per 32-token group):
  interT[d,tok]  = W^T q^T/t           (PE, accumulated over qd chunks)
  s1T[o,tok]     = objT^T interT       (PE)
  Big pass per 128-token half q_: ONE [128,512] + ONE [128,CAP-512]
  PSUM block holds 4 token groups (tile_position=(0,32j)):
      P[32j+n, c] = sum_d interT[d, 128q_+32j+n] trp[d, c] + att1p part
  where column c of group g holds an unmasked (n', a) pair.  A per-row
  window mask (host-built) zeroes other tokens' columns; ONE strided
  mod-32 reduce (DVE) yields the rotated [128, A] logits tile.  Masked
  softmax per 128-token tile (DVE+ACT exp), final einsum att2 x att1 as
  a broadcast-mult + contiguous reduce (DVE, fp16, att1 host-gathered to
  [tok, o, a_rot]).  Output [256,32] fp32 per core DMA'd out.

DMA: three queues run concurrently — SP and ACT (HWDGE) plus POOL
(SWDGE) — each carrying a byte-balanced share (~2.3 MB) of the packed
t_rep stream; small tensors issue first on ACT.  A burst of dummy
matmuls on a zeroed scratch tile during the initial DMA window keeps the
PE's HAM clock gate open (2.4 GHz instead of 1.2).
"""

import sys

if "/opt/trn_rl_repo" not in sys.path:
    sys.path.insert(0, "/opt/trn_rl_repo")

from contextlib import ExitStack

import numpy as np

import concourse.bacc as bacc
import concourse.mybir as mybir
import concourse.tile as tile
from concourse.bass_utils import run_bass_kernel_spmd

B, N, A, O, D, QD = 32, 64, 32, 32, 512, 512
NCORES = 8
BL = B // NCORES          # batches per core
TOK = BL * N              # tokens per core
NG = TOK // 32            # 32-token groups per core (8)
CAP0 = 576                # default packed-column capacity per group
F32 = mybir.dt.float32
F16 = mybir.dt.float16


def _trp_plan(cap):
    """Per-group column ranges (within the group's 4*cap flattened cols)
    for each DMA queue, byte-balanced across SP/ACT/POOL."""
    w = 4 * cap
    if cap == CAP0:
        early = {"sync": (0, 1024), "gpsimd": (1024, w)}
        late = {"sync": (0, 640), "scalar": (640, 1216), "gpsimd": (1216, w)}
        return [early] * 4 + [late] * 4
    half = (w // 2) // 64 * 64
    return [{"sync": (0, half), "gpsimd": (half, w)}] * NG


def _build(bias_over_t: float, cap: int = CAP0, reps: int = 1):
    assert cap % 64 == 0 and 576 <= cap <= 1024
    nc = bacc.Bacc("TRN2", target_bir_lowering=False, debug=False,
                   num_devices=NCORES)

    trp = nc.dram_tensor("trp", [128, NG * 4 * cap], F16,
                         kind="ExternalInput").ap()
    qT = nc.dram_tensor("qT", [QD, TOK], F16, kind="ExternalInput").ap()
    w = nc.dram_tensor("W", [QD, D], F16, kind="ExternalInput").ap()
    # objT is shipped with its O axis replicated 4x (M=128) so the s1T
    # matmul emits s1T pre-replicated into all four 32-partition bands —
    # needed because the att1p matmul's lhsT/rhs must share a start
    # partition with the band-stacked att1p layout.
    objT = nc.dram_tensor("objT", [128, 4 * BL * 128], F16,
                          kind="ExternalInput").ap()
    # att1p is stored 4 group-bands deep on the partition axis so its DMA
    # runs at full 128-partition line rate; the matmul rhs slices band
    # 32*(g%4) at a nonzero partition offset.
    att1p = nc.dram_tensor("att1p", [128, (NG // 4) * cap], F16,
                           kind="ExternalInput").ap()
    a1rot = nc.dram_tensor("a1rot", [128, 2 * O * A], F16,
                           kind="ExternalInput").ap()
    auxd = nc.dram_tensor("auxd", [128, 2 * cap], F16,
                          kind="ExternalInput").ap()
    auxm = nc.dram_tensor("auxm", [128, 2 * A], F32,
                          kind="ExternalInput").ap()
    # band mask: bmask[32k+o, tok] = 1 iff group(tok) % 4 == k.  Applied to
    # the band-replicated s1T so the att1p term can be a uniform K=128
    # matmul — the zeroed bands kill the cross-group products.
    bmask = nc.dram_tensor("bmask", [128, TOK], F16,
                           kind="ExternalInput").ap()
    out = nc.dram_tensor("out", [TOK, O], F32, kind="ExternalOutput").ap()

    plan = _trp_plan(cap)
    slabs = []
    c0 = 0
    while c0 < cap:
        slabs.append((c0, min(c0 + 512, cap)))
        c0 += 512

    with tile.TileContext(nc) as tc:
      for rep in range(reps):
       with ExitStack() as ctx:
        cpool = ctx.enter_context(tc.tile_pool(name=f"const{rep}", bufs=1))
        ppool = ctx.enter_context(tc.tile_pool(name=f"psum{rep}", bufs=2, space="PSUM"))
        lpool = ctx.enter_context(tc.tile_pool(name=f"psumL{rep}", bufs=2, space="PSUM"))
        spool = ctx.enter_context(tc.tile_pool(name=f"work{rep}", bufs=2))

        # ---- DMA: SP queue ships W+qT chunk pairs first (so the interT
        # accumulation can start as soon as chunk 0 lands), then its
        # per-group share of the packed t_rep stream.
        w_all = cpool.tile([128, 4 * D], F16, tag="w_all")
        nc.sync.dma_start(w_all[:].rearrange("p (c d) -> p c d", c=4),
                          w.rearrange("(c p) d -> p c d", p=128))
        w_sb = [w_all[:, D * c:D * (c + 1)] for c in range(4)]

        qT_all = cpool.tile([128, 4 * TOK], F16, tag="qT_all")
        nc.sync.dma_start(qT_all[:].rearrange("p (c t) -> p c t", c=4),
                          qT.rearrange("(c p) t -> p c t", p=128))
        qT_sb = [qT_all[:, TOK * c:TOK * (c + 1)] for c in range(4)]

        # Small constants issue first on the ACT queue.
        objT_all = cpool.tile([128, 4 * BL * 128], F16, tag="objT_all")
        nc.scalar.dma_start(objT_all[:], objT)
        objT_sb = [objT_all[:, BL * 128 * c:BL * 128 * (c + 1)]
                   for c in range(4)]

        att1p_sb = cpool.tile([128, (NG // 4) * cap], F16, tag="att1p_sb")
        nc.scalar.dma_start(att1p_sb[:], att1p)

        auxd_sb = cpool.tile([128, 2 * cap], F16, tag="auxd_sb")
        nc.scalar.dma_start(auxd_sb[:], auxd)

        auxm_sb = cpool.tile([128, 2 * A], F32, tag="auxm_sb")
        nc.scalar.dma_start(auxm_sb[:], auxm)
        m_sb = [auxm_sb[:, A * j:A * (j + 1)] for j in range(2)]

        a1rot_sb = cpool.tile([128, 2 * O * A], F16, tag="a1rot_sb")
        nc.scalar.dma_start(a1rot_sb[:], a1rot)
        a1rot_q = [a1rot_sb[:, O * A * j:O * A * (j + 1)] for j in range(2)]

        bmask_sb = cpool.tile([128, TOK], F16, tag="bmask_sb")
        nc.scalar.dma_start(bmask_sb[:], bmask)

        # ---- packed t_rep stream: per-group shares on all three queues
        # (issue order = group order so groups complete roughly in order).
        trp_sb = cpool.tile([128, NG * 4 * cap], F16, tag="trp_sb")
        engs = {"sync": nc.sync, "scalar": nc.scalar, "gpsimd": nc.gpsimd}
        for g in range(NG):
            gb = g * 4 * cap
            for ename, (s0, s1) in plan[g].items():
                engs[ename].dma_start(trp_sb[:, gb + s0:gb + s1],
                                      trp[:, gb + s0:gb + s1])

        # ---- PE warm-up: dummy matmuls keep the HAM clock gate open ----
        warm_sb = cpool.tile([128, 256], F16, tag="warm_sb")
        nc.vector.memset(warm_sb[:], 0)
        pscr = ppool.tile([128, 256], F32, tag="pscr", name=f"pscr_{rep}",
                          bufs=1)
        for _ in range(6):
            nc.tensor.matmul(pscr[:], warm_sb[:, :128], warm_sb[:],
                             start=True, stop=True)

        # ---- interT[d, tok] = (q/t @ W)^T, in 4 d-blocks of 128,
        # processed in m-pairs with a chunk-outer loop so PE work tracks
        # the per-chunk W/qT arrivals ----
        interT_sb = []
        for m in range(4):
            ps = ppool.tile([128, TOK], F32, tag="ps_inter")
            for c in range(4):
                nc.tensor.matmul(
                    ps[:],
                    w_sb[c][:, 128 * m:128 * (m + 1)],
                    qT_sb[c][:],
                    start=(c == 0), stop=(c == 3),
                )
            it = cpool.tile([128, TOK], F16, tag=f"interT{m}")
            nc.vector.tensor_scalar_mul(it[:], ps[:], 1.0)
            interT_sb.append(it)

        # ---- s1T[o, tok] = obj_reps . inter / t (4x band-replicated) ----
        ps1 = ppool.tile([128, TOK], F32, tag="ps_s1", bufs=1)
        for b in range(BL):
            for c in range(4):
                nc.tensor.matmul(
                    ps1[:, 64 * b:64 * (b + 1)],
                    objT_sb[c][:, 128 * b:128 * (b + 1)],
                    interT_sb[c][:, 64 * b:64 * (b + 1)],
                    start=(c == 0), stop=(c == 3),
                )
        # band-mask multiply doubles as the PSUM->SBUF f16 copy
        s1T_sb = cpool.tile([128, TOK], F16, tag="s1T")
        nc.vector.tensor_mul(s1T_sb[:], ps1[:], bmask_sb[:])

        # keep the PE busy while the first t_rep groups finish streaming
        for _ in range(2):
            nc.tensor.matmul(pscr[:], warm_sb[:, :128], warm_sb[:],
                             start=True, stop=True)

        # ---- big pass: per 128-token half, slab PSUM blocks hold 4
        # col-packed token groups (tile_position=(0,32j)) ----
        for q_ in range(2):
            mskq = spool.tile([128, cap], F16, tag="msk")
            for (c0, c1) in slabs:
                psq = lpool.tile([128, c1 - c0], F32, tag=f"psq{c0}",
                                 name=f"psq_{rep}_{q_}_{c0}")
                for c in range(4):
                    for j in range(4):
                        g = 4 * q_ + j
                        base = (g * 4 + c) * cap
                        nc.tensor.matmul(
                            psq[32 * j:32 * (j + 1), :],
                            interT_sb[c][:, 32 * g:32 * (g + 1)],
                            trp_sb[:, base + c0:base + c1],
                            start=(c == 0), stop=False,
                            tile_position=(0, 32 * j),
                            skip_group_check=True,
                        )
                for j in range(4):
                    g = 4 * q_ + j
                    half = g // 4
                    nc.tensor.matmul(
                        psq[32 * j:32 * (j + 1), :],
                        s1T_sb[:, 32 * g:32 * (g + 1)],
                        att1p_sb[:, half * cap + c0:half * cap + c1],
                        start=False, stop=True,
                        tile_position=(0, 32 * j),
                        skip_group_check=True,
                    )
                nc.vector.tensor_mul(mskq[:, c0:c1], psq[:],
                                     auxd_sb[:, q_ * cap + c0:q_ * cap + c1])
            # One strided mod-32 reduce over the masked packed columns
            # yields the [128, A] (rotated-slot) logits tile directly.
            lps = spool.tile([128, A], F32, tag="lps")
            nc.vector.reduce_sum(
                lps[:], mskq[:].rearrange("p (m a) -> p a m", a=A),
                axis=mybir.AxisListType.X,
            )

            # ---- softmax + final einsum for this 128-token tile ----
            lm = spool.tile([128, A], F32, tag="lm")
            if bias_over_t != 0.0:
                nc.vector.scalar_tensor_tensor(
                    lm[:], lps[:], bias_over_t, m_sb[q_][:],
                    op0=mybir.AluOpType.add, op1=mybir.AluOpType.mult)
            else:
                nc.vector.tensor_mul(lm[:], lps[:], m_sb[q_][:])
            negmax = spool.tile([128, 1], F32, tag="negmax")
            nc.vector.reduce_max(negmax[:], lm[:], axis=mybir.AxisListType.X,
                                 negate=True)
            e = spool.tile([128, A], F32, tag="e")
            z = spool.tile([128, 1], F32, tag="z")
            nc.scalar.activation(e[:], lm[:], mybir.ActivationFunctionType.Exp,
                                 bias=negmax[:], scale=1.0, accum_out=z[:])
            em = spool.tile([128, A], F32, tag="em")
            nc.vector.tensor_mul(em[:], e[:], m_sb[q_][:])
            ssum = spool.tile([128, 1], F32, tag="ssum")
            nc.vector.reduce_sum(ssum[:], em[:], axis=mybir.AxisListType.X)
            den = spool.tile([128, 1], F32, tag="den")
            nc.vector.tensor_scalar(
                den[:], z[:], 1e-13, ssum[:],
                op0=mybir.AluOpType.mult, op1=mybir.AluOpType.add,
            )
            rcp = spool.tile([128, 1], F32, tag="rcp")
            nc.vector.reciprocal(rcp[:], den[:])
            att2 = spool.tile([128, A], F16, tag="att2")
            nc.vector.tensor_scalar_mul(att2[:], em[:], rcp[:])

            prod = spool.tile([128, O * A], F16, tag="prod")
            nc.vector.tensor_mul(
                prod[:].rearrange("p (o a) -> p o a", o=O),
                a1rot_q[q_].rearrange("p (o a) -> p o a", o=O),
                att2[:].unsqueeze(1).broadcast_to([128, O, A]),
            )
            ot = spool.tile([128, O], F32, tag="ot")
            nc.vector.reduce_sum(
                ot[:], prod[:].rearrange("p (o a) -> p o a", o=O),
                axis=mybir.AxisListType.X,
            )
            nc.sync.dma_start(out[128 * q_:128 * (q_ + 1), :], ot[:])

    nc.compile()
    return nc


def _pack_core(t_rep_c, att1_c, tags_c, cap):
    """Pack one core's unmasked (token, a) columns group by group.

    Returns trp [128, NG*4*cap] f16, att1p [O, NG*cap] f16,
    auxd [128, 2*cap] f16, auxm [128, 2*A] f32, a1rot [128, 2*O*A] f16.
    """
    t_rep_f = t_rep_c.reshape(TOK, A, D)
    att1_f = att1_c.reshape(TOK, A, O)
    tags_f = tags_c.reshape(TOK, A) != 0

    trp = np.zeros((128, NG, 4, cap), np.float16)
    att1p = np.zeros((NG, O, cap), np.float16)
    auxd = np.zeros((2, 128, cap), np.float16)
    auxm = np.zeros((2, 128, A), np.float32)
    a1rot = np.zeros((2, 128, O, A), np.float16)
    for g in range(NG):
        col = 0
        for i in range(32):
            p = 32 * g + i
            q_, row = divmod(p, 128)
            alist = np.nonzero(tags_f[p])[0]
            cnt = len(alist)
            if cnt == 0:
                continue
            s = col
            v = t_rep_f[p, alist, :].astype(np.float16)       # [cnt, D]
            trp[:, g, :, s:s + cnt] = v.reshape(cnt, 4, 128).transpose(2, 1, 0)
            att1b = att1_f[p, alist, :]                        # [cnt, O]
            att1p[g, :, s:s + cnt] = att1b.T.astype(np.float16)
            auxd[q_, row, s:s + cnt] = 1.0
            slots = (s + np.arange(cnt)) % 32
            auxm[q_, row, slots] = 1.0
            a1rot[q_, row, :, slots] = att1b.astype(np.float16)
            col += cnt
        assert col <= cap
    # att1p device layout: partition 32*(g%4)+o, column (g//4)*cap + c
    att1p_dev = (att1p.reshape(2, 4, O, cap)        # [half, band, o, c]
                 .transpose(1, 2, 0, 3)             # [band, o, half, c]
                 .reshape(128, 2 * cap))
    return (trp.reshape(128, NG * 4 * cap),
            np.ascontiguousarray(att1p_dev),
            np.ascontiguousarray(auxd.transpose(1, 0, 2).reshape(128, 2 * cap)),
            np.ascontiguousarray(auxm.transpose(1, 0, 2).reshape(128, 2 * A)),
            np.ascontiguousarray(a1rot.transpose(1, 0, 2, 3).reshape(128, 2 * O * A)))


def _needed_cap(tags):
    per_tok = (tags.reshape(B * N, A) != 0).sum(1)
    gsum = per_tok.reshape(NCORES, NG, 32).sum(2)
    need = int(gsum.max())
    return max(CAP0, (need + 63) // 64 * 64)


def _shard_inputs(q, att1, obj_reps, tags_attention, t_rep, W, t,
                  cap=None):
    if cap is None:
        cap = _needed_cap(np.asarray(tags_attention))
    wc = np.ascontiguousarray(W, np.float16)
    bmask = np.zeros((4, O, TOK), np.float16)
    for k in range(4):
        for g in range(k, NG, 4):
            bmask[k, :, 32 * g:32 * (g + 1)] = 1.0
    bmask = np.ascontiguousarray(bmask.reshape(128, TOK))
    in_maps = []
    for i in range(NCORES):
        bs = slice(BL * i, BL * (i + 1))
        qf = (q[bs, :, 0, :].reshape(TOK, QD) / float(t)).astype(np.float16)
        objd = obj_reps[bs].transpose(0, 2, 1).astype(np.float16)  # [b, d, o]
        trp, att1p, auxd, auxm, a1rot = _pack_core(
            np.asarray(t_rep[bs], np.float32),
            np.asarray(att1[bs], np.float32),
            np.asarray(tags_attention[bs]), cap)
        in_maps.append({
            "trp": trp,
            "qT": np.ascontiguousarray(qf.T),
            "W": wc,
            "objT": np.ascontiguousarray(
                np.tile(objd.reshape(BL, 4, 128, O)
                        .transpose(2, 1, 0, 3), (1, 1, 1, 4))
                .reshape(128, 4 * BL * 128)),
            "att1p": att1p,
            "a1rot": a1rot,
            "auxd": auxd,
            "auxm": auxm,
            "bmask": bmask,
        })
    return in_maps


_NC_CACHE = {}


def _get_nc(bias_over_t: float, cap: int = CAP0, reps: int = 1):
    key = (float(bias_over_t), int(cap), int(reps))
    if key not in _NC_CACHE:
        _NC_CACHE[key] = _build(key[0], cap=key[1], reps=key[2])
    return _NC_CACHE[key]


def _run(inputs, trace=False, **kw):
    q = np.asarray(inputs["q"], np.float32)
    att1 = np.asarray(inputs["att1"], np.float32)
    obj_reps = np.asarray(inputs["obj_reps"], np.float32)
    tags = np.asarray(inputs["tags_attention"])
    t_rep = np.asarray(inputs["t_rep"], np.float32)
    W = np.asarray(inputs["W"], np.float32)
    bias = float(np.asarray(inputs["bias"]))
    t = float(np.asarray(inputs["t"]))

    cap = _needed_cap(tags)
    nc = _get_nc(bias / t, cap=cap)
    in_maps = _shard_inputs(q, att1, obj_reps, tags, t_rep, W, t, cap=cap)
    res = run_bass_kernel_spmd(nc, in_maps, core_ids=list(range(NCORES)),
                               trace=trace, **kw)
    outs = [np.asarray(res.results[i]["out"]).reshape(BL, N, O)
            for i in range(NCORES)]
    full = np.concatenate(outs, axis=0)
    return full, res


def kernel(**inputs):
    full, _ = _run(inputs, trace=False)
    return full
